# revision 22
# baseline (speedup 1.0000x reference)
"""Trainium2 Bass kernel for the 1x1-conv attention module.

Shapes (hardcoded): x (8, 64, 64, 64) fp32, w_qkv (192, 64), b_qkv (192,),
w_out (64, 64), b_out (64,). Data-parallel: one batch element per NeuronCore
(8 cores). Channel-major layout (c on partitions, t = h*64+w on the free
dim); the reference's view/permute quirk composes to the standard
channel-major permute, so no data movement is needed for it.

v2 pipeline (fp8 PV + two-engine exp):
  QKV projections on TensorE in fp32r (K=65: a ones row of x absorbs the
  biases; softmax scale folded into w_q host-side).  q/k duplicated into
  both partition halves so QK^T row-packs two K=64 j-tiles.  Scores are
  computed transposed (s_T[j, i]) in 512-query i-chunks, 16 waves of 2
  j-tiles per chunk (2-bank PSUM slots, double-buffered).

  exp() is SPLIT across two engines: ~9/16 waves run on ScalarE
  (activation Exp, bias -2.5 folded in, writing float8e4 directly) and
  ~7/16 on VectorE via a Schraudolph/Mitchell bit-trick -- y = z*A + B
  converted to uint8 with saturating round IS the fp8e4m3 encoding of
  exp(z - 2.5) to ~5% RMS.  The shared bias cancels in softmax
  normalization; fp8 quantization noise averages out over 4096 keys.

  PV runs in fp8 DoubleRow mode: stationary [128, 2, 128] packs V for two
  j-tiles (v channels | ones col | zero pad), moving is the fp8 exp pair
  [128, 2, 256] -- 2x fp8 rate and the rowsum rides along at out
  partition 64, so PV+rowsum cost 4096 PE cycles/chunk vs 16384 for the
  fp32r version.  Emission is block-pipelined (6 waves/block): QK for the
  block, both engines' exps, then the PREVIOUS block's PV matmuls, so PV
  never blocks the exp chain; normalization (reciprocal + K=1 broadcast
  matmul + multiply) and the output projection + residual run as a
  3-stage lagged tail through a dedicated 2-bank PSUM pool.
"""

import numpy as np

B, C, HW = 8, 64, 4096
NCORES = 8
IC = 512  # queries per i-chunk
NIC = HW // IC  # 8
NJ = HW // 128  # 32 j-tiles of 128 tokens
NW = NJ // 2  # 16 waves (= DoubleRow pairs) per chunk

EXP_BIAS = -2.5
SCH_A = 8 * 1.4426950408889634  # 8/ln2
SCH_B = SCH_A * EXP_BIAS + 56.0 - 0.344

_compiled = None


def _build_bass(repeat=1, act_waves=(0, 1, 3, 5, 7, 9, 11, 13, 15),
                do_exp=True, do_av=True, do_norm=True, pipelined=True,
                epool_bufs=14, u_bufs=1, s_bufs=3, blocks=(6, 6, 4), dbg=False,
                tail_mask=7, do_v=True, qk_nopack=False, exp_dt="f8",
                sched="wave", pv_lag=7):
    import concourse.bass as bass
    import concourse.mybir as mybir
    import concourse.tile as tile

    FP = mybir.dt.float32
    FR = mybir.dt.float32r
    F8 = mybir.dt.float8e4
    U8 = mybir.dt.uint8
    I32 = mybir.dt.int32
    Exp = mybir.ActivationFunctionType.Exp
    Alu = mybir.AluOpType
    DR = mybir.MatmulPerfMode.DoubleRow
    act_set = frozenset(act_waves)
    assert sum(blocks) == NW

    nc = bass.Bass("TRN2", target_bir_lowering=False, debug=False)

    xa_d = nc.dram_tensor("xa", [C + 1, HW], FP, kind="ExternalInput")
    xb_d = nc.dram_tensor("xb", [C, HW], FP, kind="ExternalInput")
    wq_d = nc.dram_tensor("wq", [C + 1, C], FP, kind="ExternalInput")
    wk_d = nc.dram_tensor("wk", [C + 1, C], FP, kind="ExternalInput")
    wv_d = nc.dram_tensor("wv", [C + 1, C], FP, kind="ExternalInput")
    wo_d = nc.dram_tensor("wo", [C, C], FP, kind="ExternalInput")
    out_d = nc.dram_tensor("out", [C, HW], FP, kind="ExternalOutput")
    if dbg:
        dbg_e_d = nc.dram_tensor("dbg_e", [128, NJ, IC], U8, kind="ExternalOutput")
        dbg_u_d = nc.dram_tensor("dbg_u", [128, IC], FP, kind="ExternalOutput")
        dbg_rb_d = nc.dram_tensor("dbg_rb", [C, IC], FP, kind="ExternalOutput")

    with tile.TileContext(nc) as tc:
        with (
            nc.allow_low_precision(reason="fp8 scores/V (fp32 accum in PSUM)"),
            tc.tile_pool(name="singles", bufs=1) as singles,
            tc.tile_pool(name="escr", bufs=epool_bufs) as epool,
            tc.tile_pool(name="att", bufs=2) as apool,
            tc.tile_pool(name="rbp", bufs=2) as rbpool,
            tc.tile_pool(name="outp", bufs=2) as opool,
            tc.tile_pool(name="sps", bufs=s_bufs, space="PSUM") as spool,
            tc.tile_pool(name="ups", bufs=u_bufs, space="PSUM") as upool,
            tc.tile_pool(name="tps", bufs=1, space="PSUM") as tpool,
        ):
            # ---- load inputs ----
            xa = singles.tile([C + 1, HW], FP)
            xb = singles.tile([C, HW], FP)
            wq = singles.tile([C + 1, C], FP)
            wk = singles.tile([C + 1, C], FP)
            wv = singles.tile([C + 1, C], FP)
            wo = singles.tile([C, C], FP)
            nc.sync.dma_start(out=xa[:], in_=xa_d[:])
            nc.sync.dma_start(out=xb[:], in_=xb_d[:])
            nc.sync.dma_start(out=wq[:], in_=wq_d[:])
            nc.sync.dma_start(out=wk[:], in_=wk_d[:])
            nc.sync.dma_start(out=wv[:], in_=wv_d[:])
            nc.sync.dma_start(out=wo[:], in_=wo_d[:])

            # fp32r copies (walrus requires matmul inputs produced
            # rounded-to-fp32r by an engine op)
            xar = singles.tile([C + 1, HW], FR)
            wqr = singles.tile([C + 1, C], FR)
            wkr = singles.tile([C + 1, C], FR)
            wvr = singles.tile([C + 1, C], FR)
            wor = singles.tile([C, C], FR)
            nc.vector.tensor_copy(xar[:], xa[:])
            nc.vector.tensor_copy(wqr[:], wq[:])
            nc.vector.tensor_copy(wkr[:], wk[:])
            nc.vector.tensor_copy(wvr[:], wv[:])
            nc.vector.tensor_copy(wor[:], wo[:])

            ones32 = singles.tile([128, 1], FP)
            nc.vector.memset(ones32[:], 1.0)
            bias_t = singles.tile([128, 1], FP)
            nc.vector.memset(bias_t[:], EXP_BIAS)
            # preload the exp table set while DMAs are in flight
            expwarm = singles.tile([1, 1], FP)
            nc.scalar.activation(expwarm[:], ones32[0:1, :], Exp)

            # q, k channel-major, duplicated into both partition halves
            qd = singles.tile([128, HW], FR)
            kd = singles.tile([128, HW], FR)
            # fp8 DoubleRow stationary: [j(128), pair, half, col] where
            # col = v channels (64) | ones (64) -- the 64 ones columns make
            # PSUM partitions 64..127 all carry the rowsum, so one wide DVE
            # reciprocal yields the already-broadcast 1/rowsum in SBUF.
            vt8 = singles.tile([128, NW, 2, 128], F8)
            nc.gpsimd.memset(vt8[:], 1.0)

            def emit_kq(dst, w_, n, eng):
                sl = slice(n * IC, (n + 1) * IC)
                p = spool.tile([C, IC], FP, tag="s", name="pkq")
                nc.tensor.matmul(p[:], w_[:], xar[:, sl], start=True, stop=True)
                if eng == "act":
                    nc.scalar.copy(dst[0:C, sl], p[:])
                else:
                    nc.vector.tensor_copy(dst[0:C, sl], p[:])
                nc.sync.dma_start(out=dst[C:128, sl], in_=dst[0:C, sl])

            def emit_v_group(g):
                # 8 token-tiles (= 4 pairs) per PSUM bank
                p = spool.tile([128, 8, C], FP, tag="s", name="pvg")
                for t in range(8):
                    jc = g * 8 + t
                    jsl = slice(jc * 128, (jc + 1) * 128)
                    nc.tensor.matmul(
                        p[:, t, :], xar[:, jsl], wvr[:], start=True, stop=True
                    )
                nc.vector.tensor_copy(
                    vt8[:, 4 * g : 4 * (g + 1), :, 0:C],
                    p[:].rearrange("p (a b) c -> p a b c", a=4),
                )

            for _rep in range(repeat):
                # ---- projections ----
                if sched == "wave":
                    # minimal upfront; the rest streams through the wave loop
                    emit_kq(kd, wkr, 0, "act")
                    emit_kq(kd, wkr, 1, "act")
                    emit_kq(qd, wqr, 0, "dve")
                    stream = (
                        [lambda n=n: emit_kq(kd, wkr, n, "act") for n in (2, 3, 4)]
                        + ([lambda: emit_v_group(0)] if do_v else [])
                        + [lambda: emit_kq(kd, wkr, 5, "act")]
                        + ([lambda: emit_v_group(1)] if do_v else [])
                        + [lambda: emit_kq(kd, wkr, 6, "act")]
                        + ([lambda: emit_v_group(2)] if do_v else [])
                        + [lambda: emit_kq(kd, wkr, 7, "act")]
                        + ([lambda: emit_v_group(3)] if do_v else [])
                        + [lambda n=n: emit_kq(qd, wqr, n, "dve") for n in range(1, NIC)]
                    )
                else:
                    for n in range(NIC):
                        emit_kq(kd, wkr, n, "act")
                    emit_kq(qd, wqr, 0, "dve")
                    if do_v:
                        for g in range(NJ // 8):
                            emit_v_group(g)
                    for n in range(1, NIC):
                        emit_kq(qd, wqr, n, "dve")
                    stream = []

                # ---- main loop ----
                tail_q = []  # pending lagged tail stages (closures)

                def emit_tail(tic, tu):
                    tsl = slice(tic * IC, (tic + 1) * IC)
                    st = {}

                    def t0():
                        # wide reciprocal: partitions 64..127 of u all hold
                        # the rowsum -> [C, IC] of broadcast 1/rowsum
                        if not tail_mask & 1:
                            return
                        # fast inverse: bits(1/x) ~= C0 - bits(x), done as
                        # ~x + (C0+1) in one int32 tensor_scalar (the DVE
                        # reciprocal instruction costs ~14x a plain ALU op)
                        rb = rbpool.tile([C, IC], FR, name="rb")
                        nc.vector.tensor_scalar(
                            rb[:].bitcast(I32),
                            tu[C : 2 * C, :].bitcast(I32),
                            -1,
                            0x7EF311C3,
                            Alu.mult,
                            Alu.add,
                        )
                        st["rb"] = rb
                        if dbg and tic == 0 and _rep == 0:
                            du = opool.tile([128, IC], FP, name="du")
                            nc.vector.tensor_copy(du[:], tu[:])
                            nc.sync.dma_start(out=dbg_u_d[:], in_=du[:])
                            nc.sync.dma_start(
                                out=dbg_rb_d[:], in_=rb[:].bitcast(FP)
                            )

                    def t1():
                        if not tail_mask & 2:
                            return
                        att = apool.tile([C, IC], FR, name="att")
                        nc.vector.tensor_mul(att[:], tu[0:C, :], st["rb"])
                        po = tpool.tile([C, IC], FP, tag="t", name="po")
                        nc.tensor.matmul(
                            po[:], wor[:], att[:], start=True, stop=True
                        )
                        st["po"] = po

                    def t2():
                        if not tail_mask & 4:
                            return
                        o = opool.tile([C, IC], FP, name="o")
                        nc.vector.tensor_add(o[:], st["po"], xb[:, tsl])
                        nc.sync.dma_start(out=out_d[:, tsl], in_=o[:])

                    return [t0, t1, t2]

                def emit_qk(ic, w):
                    isl = slice(ic * IC, (ic + 1) * IC)
                    s = spool.tile([128, 2, IC], FP, tag="s", name="s")
                    for t in range(2):
                        j = 2 * w + t
                        jh = 0 if qk_nopack else j % 2
                        hsl = slice(64 * jh, 64 * (jh + 1))
                        nc.tensor.matmul(
                            s[:, t, :],
                            kd[hsl, j * 128 : (j + 1) * 128],
                            qd[hsl, isl],
                            start=True,
                            stop=True,
                        )
                    return s

                def emit_exp(ic, u, w, s):
                    e3p = epool.tile(
                        [128, 2, IC], F8 if exp_dt == "f8" else FP,
                        name="e3p",
                    )
                    if do_exp:
                        if w in act_set:
                            nc.scalar.activation(
                                e3p[:], s[:], Exp, bias=bias_t[:]
                            )
                        else:
                            nc.vector.tensor_scalar(
                                e3p[:].bitcast(U8)
                                if exp_dt == "f8"
                                else e3p[:],
                                s[:],
                                SCH_A,
                                SCH_B,
                                Alu.mult,
                                Alu.add,
                            )
                    if dbg and ic == 0 and _rep == 0:
                        nc.sync.dma_start(
                            out=dbg_e_d[:, 2 * w : 2 * w + 2, :],
                            in_=e3p[:].bitcast(U8),
                        )
                    return (ic, u, w, e3p)

                def emit_pv(pic, pu, pw, pe):
                    for h in range(2):
                        hs = slice(h * 256, (h + 1) * 256)
                        nc.tensor.matmul(
                            pu[:, hs],
                            vt8[:, pw, :, :],
                            pe[:, :, hs],
                            # start zeroes the whole 2KB bank (HW
                            # zero-region): only the first write may
                            # set it, or it erases the other half
                            start=(pw == 0 and h == 0),
                            stop=(pw == NW - 1 and h == 1),
                            perf_mode=DR,
                            skip_group_check=True,
                        )
                    if pw == NW - 1 and do_norm:
                        tail_q.extend(emit_tail(pic, pu))

                if sched == "wave":
                    pend = []
                    for ic in range(NIC):
                        u = upool.tile([128, IC], FP, tag="u", name="u")
                        for w in range(NW):
                            # tail DVE ops pop BEFORE this wave's QK/exp so
                            # they sit ahead of exp(w) in the DVE queue --
                            # with u_bufs=1 the next chunk's first PV waits
                            # on them (deadlock-free by construction)
                            for _ in range(2):
                                if tail_q:
                                    tail_q.pop(0)()
                            s = emit_qk(ic, w)
                            if stream:
                                stream.pop(0)()
                            if do_av and len(pend) >= pv_lag:
                                emit_pv(*pend.pop(0))
                            pend.append(emit_exp(ic, u, w, s))
                    while pend:
                        if do_av:
                            emit_pv(*pend.pop(0))
                        else:
                            pend.pop(0)
                        if tail_q:
                            tail_q.pop(0)()
                    while tail_q:
                        tail_q.pop(0)()
                else:
                    prev = []  # deferred exp results awaiting PV
                    for ic in range(NIC):
                        u = upool.tile([128, IC], FP, tag="u", name="u")
                        wbase = 0
                        for nb in blocks:
                            bw = list(range(wbase, wbase + nb))
                            wbase += nb
                            slots = [(w, emit_qk(ic, w)) for w in bw]
                            if tail_q:
                                tail_q.pop(0)()
                            cur = [emit_exp(ic, u, w, s) for w, s in slots]
                            if do_av:
                                if pipelined:
                                    todo, prev = prev, cur
                                else:
                                    todo = cur
                                for item in todo:
                                    emit_pv(*item)
                    if do_av and pipelined:
                        for item in prev:
                            emit_pv(*item)
                    while tail_q:
                        tail_q.pop(0)()

    _split_matmul_waits(nc, mybir)
    return nc
